# revision 37
# baseline (speedup 1.0000x reference)
"""Capsule-routing kernel for 8 Trainium2 NeuronCores — v2.

Problem: u_hat = einsum('nidk,bik->bnid', W, x); 3 rounds of dynamic
routing (softmax over n, weighted sum over i, squash, agreement update).

Sharding: input-capsule axis i (2048) split 8 ways -> 256 i per core.

v2 design (vs baseline 818us):
  - s0 (iteration-0 weighted sum, uniform c) computed DIRECTLY from x,W
    on PE: s0[b,(d,n)] = sum_{i,k} x8[(i8,k),b] * W[(i8,k),(d,n)],
    accumulated over 32 group-pairs in one psum tile. This makes out0
    available ~50us in, long before u_hat materialization completes.
  - W is streamed twice (pass 1 for s0, pass 2 for u_hat); u_hat is kept
    fp16 SBUF-resident for the first RES_P pairs, spilled to DRAM for
    the rest (store once, reload per routing sweep).
  - Sweep-1 u_hat production is software-pipelined WITH routing iter 1
    in a single merged emission loop (per-engine program order matters);
    iter-1 consumes pairs in production order.
  - psum drains rotate over ACT/GPSIMD/DVE; the d-reduction tree is
    split GPS(d32->4)/DVE(d4->1); Z comes free from exp via ACT
    accum_out; e is broadcast (stride-0 middle axis) into the sm-mul so
    no crep materialization.
  - Layouts: u16 partition p = 32*j + b (j = i mod 4 in group), free
    (d,n) d-major. Fold over the 4 j-slots + 1/Z via cz4 ones-matmul
    accumulated across all 64 groups in one psum tile (baseline trick).
"""
import sys
import types
from collections import defaultdict

sys.path.insert(0, "/opt/trn_rl_repo")

import numpy as np

from concourse import bacc, tile, mybir
from concourse.bass_utils import run_bass_kernel_spmd

f32 = mybir.dt.float32
f16 = mybir.dt.float16
AX = mybir.AxisListType
OP = mybir.AluOpType
AF = mybir.ActivationFunctionType

B, N, I, D, K = 32, 64, 2048, 32, 16
NCORES = 8
IL = I // NCORES          # 256 local input capsules
G = IL // 4               # 64 groups of 4 i
NP = G // 2               # 32 group-pairs
DN = D * N                # 2048 free elements per group, d-major
INV_LOG2 = float(1.0 / np.log(2.0))

RES_P = 0                # pairs kept SBUF-resident; rest spilled to DRAM
LAG = 6                   # mega-loop emission lag: iter1 pair q at step q+LAG


def _install_ntff_hook():
    if "antenv.axon_hooks" in sys.modules:
        return
    try:
        mod = types.ModuleType("antenv.axon_hooks")
        state = {"hook": None}
        mod.set_axon_ntff_profile_hook = lambda h: state.__setitem__("hook", h)
        mod.get_axon_ntff_profile_hook = lambda: state["hook"]
        sys.modules["antenv.axon_hooks"] = mod
        import antenv
        antenv.axon_hooks = mod
        from trn_agent_boot.trn_boot import _ntff_profile_via_ctypes
        mod.set_axon_ntff_profile_hook(
            _ntff_profile_via_ctypes("/opt/axon/libaxon_pjrt.so"))
    except Exception:
        pass


def _build():
    nc = bacc.Bacc("TRN2", target_bir_lowering=False, debug=False,
                   num_devices=NCORES)

    w_t4 = nc.dram_tensor("w_t4", [NP // 4, 128, 4 * DN], f16,
                          kind="ExternalInput")
    x_bd = nc.dram_tensor("x_bd", [128, NP, 128], f16, kind="ExternalInput")
    x8 = nc.dram_tensor("x8", [128, NP, B], f16, kind="ExternalInput")
    s2_part = nc.dram_tensor("s2_part", [B, DN], f32, kind="ExternalOutput")

    u_spill = nc.dram_tensor("u_spill", [NP, 128, 2 * DN], f16)
    cc_in = [nc.dram_tensor(f"cc_in{r}", [B, DN], f16) for r in range(2)]
    cc_out = [nc.dram_tensor(f"cc_out{r}", [B, DN], f16, addr_space="Shared")
              for r in range(2)]

    ones4_np = np.zeros((128, 32), np.float16)
    for p in range(128):
        ones4_np[p, p % 32] = 1.0
    ones4 = nc.inline_tensor(ones4_np, name="ones4")

    core_ids = list(range(NCORES))

    with tile.TileContext(nc) as tc:
        with tc.tile_pool(name="const", bufs=1) as constp, \
             tc.tile_pool(name="tail", bufs=1) as tail, \
             tc.tile_pool(name="xw", bufs=1) as xw, \
             tc.tile_pool(name="wq", bufs=3) as wq, \
             tc.tile_pool(name="ures", bufs=1) as ures, \
             tc.tile_pool(name="spo", bufs=2) as spo, \
             tc.tile_pool(name="win", bufs=3) as win, \
             tc.tile_pool(name="tmpp", bufs=2) as tmpp, \
             tc.tile_pool(name="smp", bufs=2) as smp, \
             tc.tile_pool(name="tr1p", bufs=2) as tr1p, \
             tc.tile_pool(name="tr2p", bufs=2) as tr2p, \
             tc.tile_pool(name="tr3p", bufs=2) as tr3p, \
             tc.tile_pool(name="t4p", bufs=2) as t4p, \
             tc.tile_pool(name="bstate", bufs=1) as bstate, \
             tc.tile_pool(name="small", bufs=4) as small, \
             tc.tile_pool(name="psu", bufs=2, space="PSUM") as psu, \
             tc.tile_pool(name="psacc", bufs=1, space="PSUM") as psacc:

            ones_sb = constp.tile([128, 32], f16)
            nc.sync.dma_start(ones_sb[:], ones4[:])
            xbd_sb = xw.tile([128, NP, 128], f16)
            nc.sync.dma_start(xbd_sb[:], x_bd[:])
            x8_sb = xw.tile([128, NP, B], f16)
            nc.scalar.dma_start(x8_sb[:], x8[:])
            orep = [constp.tile([128, DN], f16, tag="orep", name=f"orep{r}")
                    for r in range(2)]

            def squash_to_outrep(s_sb, orep_t, pre_scale):
                """orep [128,(d,n)] f16 <- x4-replicated squash(s_sb*pre_scale)."""
                ps2 = float(pre_scale * pre_scale)
                sq = tail.tile([32, D, N], f32, tag="t_sq")
                nc.scalar.square(sq[:],
                                 s_sb[:].rearrange("p (d n) -> p d n", n=N))
                cur, d = sq, D
                while d > 2:
                    nxt = tail.tile([32, d // 2, N], f32, tag=f"t_tr{d}")
                    nc.vector.tensor_add(nxt[:], cur[:, 0:d // 2, :],
                                         cur[:, d // 2:d, :])
                    cur, d = nxt, d // 2
                sn = tail.tile([32, 1, N], f32, tag="t_sn")
                nc.vector.tensor_add(sn[:], cur[:, 0:1, :], cur[:, 1:2, :])
                r_ = tail.tile([32, N], f32, tag="t_r")
                nc.scalar.activation(r_[:], sn[:, 0, :], AF.Sqrt,
                                     bias=0.0, scale=ps2)
                den = tail.tile([32, N], f32, tag="t_den")
                nc.vector.tensor_scalar(den[:], sn[:, 0, :], ps2, 1.0,
                                        OP.mult, OP.add)
                rd = tail.tile([32, N], f32, tag="t_rd")
                nc.vector.reciprocal(rd[:], den[:])
                fac = tail.tile([32, N], f32, tag="t_fac")
                nc.vector.scalar_tensor_tensor(fac[:], r_[:],
                                               float(pre_scale * INV_LOG2),
                                               rd[:],
                                               op0=OP.mult, op1=OP.mult)
                frep = tail.tile([32, D, N], f16, tag="t_frep")
                nc.scalar.copy(frep[:, 0:1, :], fac[:].unsqueeze(1))
                d = 1
                while d < D:
                    nc.scalar.copy(frep[:, d:2 * d, :], frep[:, 0:d, :])
                    d *= 2
                o16 = tail.tile([32, D, N], f16, tag="t_o16")
                nc.vector.tensor_mul(
                    o16[:], s_sb[:].rearrange("p (d n) -> p d n", n=N), frep[:])
                for j in range(4):
                    nc.sync.dma_start(
                        orep_t[32 * j:32 * j + 32, :],
                        o16[:].rearrange("p d n -> p (d n)"))

            def dma4(dst, src, n=4):
                # split a large DMA along the partition axis so it spreads
                # over n queues instead of serializing on one (~22GB/s each)
                p = dst.shape[0]
                c = p // n
                for a in range(n):
                    nc.sync.dma_start(dst[c * a:c * (a + 1)],
                                      src[c * a:c * (a + 1)])

            # =========== pass 1: s0 directly from x,W ===========
            s0_ps = psacc.tile([B, DN], f32, tag="sacc")
            p1_w = {}
            for t in range(NP // 4):
                wt = wq.tile([128, 4 * DN], f16, tag="wt")
                if t >= NP // 4 - 2:
                    p1_w[t] = wt
                if t % 2 == 0:
                    nc.scalar.dma_start(wt[:], w_t4[t])
                else:
                    nc.sync.dma_start(wt[:], w_t4[t])
                for j in range(4):
                    gp = 4 * t + j
                    for ch in range(4):
                        nc.tensor.matmul(
                            s0_ps[:, 512 * ch:512 * (ch + 1)],
                            lhsT=x8_sb[:, gp, :],
                            rhs=wt[:, 2048 * j + 512 * ch:
                                   2048 * j + 512 * (ch + 1)],
                            start=(gp == 0), stop=(gp == NP - 1),
                            skip_group_check=True)
            s0_dr = tail.tile([B, DN], f16, tag="t_io16")
            nc.scalar.copy(s0_dr[:], s0_ps[:])
            nc.sync.dma_start(cc_in[0][:], s0_dr[:])
            nc.gpsimd.collective_compute(
                "AllReduce", OP.add, ins=[cc_in[0][:]],
                outs=[cc_out[0][:]], replica_groups=[core_ids])
            s0_all = tail.tile([B, DN], f16, tag="t_io16")
            nc.sync.dma_start(s0_all[:], cc_out[0][:])
            squash_to_outrep(s0_all, orep[0], 1.0 / 64.0)

            # =========== merged: u_hat production (pass 2) + iter 1 ===========
            # production order: reuse pass-1's still-resident last-6 W tiles
            # first (zero W-DMA right after pass 1 ends), then stream the rest
            N_REUSE = 8
            PROD = list(range(NP - N_REUSE, NP)) + list(range(NP - N_REUSE))
            RES_SET = set(PROD[:RES_P])
            pair_tiles = {}    # pair id -> resident sbuf tile [128, 2, DN]
            bs_tiles = {}

            def drain(dst, src):
                nc.scalar.copy(dst, src)

            wq_tiles = {}

            def emit_produce(gp):
                """u_hat matmuls + drains for pair gp (quad W tile)."""
                t, j = gp // 4, gp % 4
                if t in p1_w:
                    wt = p1_w[t]
                elif t in wq_tiles:
                    wt = wq_tiles[t]
                else:
                    wt = wq.tile([128, 4 * DN], f16, tag="wt")
                    wq_tiles[t] = wt
                    nc.sync.dma_start(wt[:], w_t4[t])
                if gp in RES_SET:
                    pt = ures.tile([128, 2, DN], f16, tag=f"ur{gp}",
                                   name=f"ur{gp}")
                else:
                    pt = spo.tile([128, 2, DN], f16, tag="spo")
                pair_tiles[gp] = pt
                for gs in range(2):
                    for h in range(2):
                        pu = psu.tile([128, DN // 2], f32)
                        for ch in range(2):
                            nc.tensor.matmul(
                                pu[:, 512 * ch:512 * (ch + 1)],
                                lhsT=xbd_sb[64 * gs:64 * (gs + 1), gp, :],
                                rhs=wt[64 * gs:64 * (gs + 1),
                                       2048 * j + 1024 * h + 512 * ch:
                                       2048 * j + 1024 * h + 512 * (ch + 1)],
                                start=True, stop=True)
                        drain(pt[:, gs, 1024 * h:1024 * (h + 1)], pu[:])
                if gp not in RES_SET:
                    nc.sync.dma_start(
                        u_spill[gp], pt[:].rearrange("p a f -> p (a f)"))

            def load_pair(q):
                """DRAM -> win tile for spilled pair q."""
                pt = win.tile([128, 2, DN], f16, tag="win")
                if q % 2 == 0:
                    nc.scalar.dma_start(
                        pt[:].rearrange("p a f -> p (a f)"), u_spill[q])
                else:
                    nc.sync.dma_start(
                        pt[:].rearrange("p a f -> p (a f)"), u_spill[q])
                return pt

            # per-pair iter state carried between pipeline stages
            st = {}

            def it_stage_a(q, u_pt, orep_t):
                """tmp mul + DVE tree stages (d 32 -> 4). Plain tensor_tensor
                runs at the 2x DVE rate for packed fp16 — measured faster
                than both GPSIMD (fixed ~3us/op) and STT (~1.3x)."""
                u3 = u_pt[:]                                  # [128, 2, 2048]
                orep3 = orep_t[:].unsqueeze(1).broadcast_to([128, 2, DN])
                tmp = tmpp.tile([128, 2, DN], f16, tag="tmp")
                nc.vector.tensor_mul(tmp[:], u3, orep3)
                t1 = tr1p.tile([128, 2, 16 * N], f16, tag="t1")
                nc.vector.tensor_add(t1[:], tmp[:, :, 0:16 * N],
                                     tmp[:, :, 16 * N:32 * N])
                t2 = tr2p.tile([128, 2, 8 * N], f16, tag="t2")
                nc.vector.tensor_add(t2[:], t1[:, :, 0:8 * N],
                                     t1[:, :, 8 * N:16 * N])
                t3 = tr3p.tile([128, 2, 4 * N], f16, tag="t3")
                nc.vector.tensor_add(t3[:], t2[:, :, 0:4 * N],
                                     t2[:, :, 4 * N:8 * N])
                st[q] = {"u3": u3, "t3": t3}

            def it_stage_b(q, first_iter):
                """DVE tree tail + bs update + max; GPS nm2; ACT exp+Z."""
                t3 = st[q]["t3"]
                t4 = t4p.tile([128, 2, 2 * N], f16, tag="t4")
                nc.vector.tensor_add(t4[:], t3[:, :, 0:2 * N],
                                     t3[:, :, 2 * N:4 * N])
                if first_iter:
                    bs = bstate.tile([128, 2, N], f32, tag=f"bs{q}",
                                     name=f"bs{q}")
                    bs_tiles[q] = bs
                    nc.vector.tensor_add(bs[:], t4[:, :, 0:N], t4[:, :, N:2 * N])
                else:
                    bs = bs_tiles[q]
                    a2 = small.tile([128, 2, N], f32, tag="a2")
                    nc.vector.tensor_add(a2[:], t4[:, :, 0:N], t4[:, :, N:2 * N])
                    nc.vector.tensor_add(bs[:], bs[:], a2[:])
                nm2 = small.tile([128, 2, 1], f32, tag="nm2")
                nc.vector.tensor_reduce(out=nm2[:], in_=bs[:], axis=AX.X,
                                        op=OP.max, negate=True)
                e2 = small.tile([128, 2, N], f16, tag="e2")
                for gs in range(2):
                    nc.scalar.activation(e2[:, gs, :], bs[:, gs, :], AF.Exp,
                                         bias=nm2[:, gs, :], scale=1.0)
                st[q].update(e2=e2)

            def it_stage_c(q, s_ps, first_mm, last_mm):
                """DVE rz + sm mul; ACT cz4; PE folds."""
                d = st.pop(q)
                z2 = small.tile([128, 2, 1], f32, tag="z2")
                nc.vector.tensor_reduce(out=z2[:], in_=d["e2"][:], axis=AX.X,
                                        op=OP.add)
                rz = small.tile([128, 2, 1], f32, tag="rz")
                nc.vector.reciprocal(rz[:], z2[:])
                cz4 = small.tile([128, 2, 32], f16, tag="cz4")
                for gs in range(2):
                    nc.scalar.activation(cz4[:, gs, :], ones_sb[:], AF.Copy,
                                         bias=0.0, scale=rz[:, gs, :])
                sm = smp.tile([128, 2, DN], f16, tag="sm")
                u4 = d["u3"].rearrange("p a (d n) -> p a d n", n=N)
                e4 = d["e2"][:].unsqueeze(2).broadcast_to([128, 2, D, N])
                nc.vector.tensor_mul(
                    sm[:].rearrange("p a (d n) -> p a d n", n=N), u4, e4)
                smf = sm[:]
                return (cz4, smf, first_mm, last_mm, s_ps)

            def emit_fold(fold_args):
                cz4, smf, first_mm, last_mm, s_ps = fold_args
                for gs in range(2):
                    for ch in range(4):
                        nc.tensor.matmul(
                            s_ps[:, 512 * ch:512 * (ch + 1)],
                            lhsT=cz4[:, gs, :],
                            rhs=smf[:, gs, 512 * ch:512 * (ch + 1)],
                            start=(first_mm and gs == 0),
                            stop=(last_mm and gs == 1),
                            skip_group_check=True)

            # fold emission slots: keep PE stream aligned with real time
            fold_slot = {}
            for q in range(NP):
                fold_slot[q] = max(q + LAG + 2, 9 + (3 * q) // 2)
            slot_to_q = defaultdict(list)
            for q, s in fold_slot.items():
                slot_to_q[s].append(q)

            s1_ps = psacc.tile([B, DN], f32, tag="sacc")
            pending_folds = {}
            it1_tiles = {}
            it1_done = set()
            n_steps = max(NP, max(fold_slot.values()) + 1)
            for step in range(n_steps):
                pa = step - LAG
                if 0 <= pa < NP:
                    for wpos in range(pa, min(pa + 3, NP)):
                        w = PROD[wpos]
                        if wpos >= RES_P and w not in it1_done \
                                and w not in it1_tiles:
                            it1_tiles[w] = load_pair(w)
                    qa = PROD[pa]
                    if pa < RES_P:
                        u_pt = pair_tiles[qa]
                    else:
                        u_pt = it1_tiles.pop(qa)
                        it1_done.add(qa)
                    it_stage_a(qa, u_pt, orep[0])
                pb = step - LAG - 1
                if 0 <= pb < NP:
                    it_stage_b(PROD[pb], first_iter=True)
                pc = step - LAG - 2
                if 0 <= pc < NP:
                    pending_folds[pc] = it_stage_c(
                        PROD[pc], s1_ps, first_mm=(pc == 0),
                        last_mm=(pc == NP - 1))
                if step < NP:
                    emit_produce(PROD[step])
                for p_ in slot_to_q.get(step, ()):
                    emit_fold(pending_folds.pop(p_))

            # iter-1 tail: AllReduce + squash -> orep1
            s1_dr = tail.tile([B, DN], f16, tag="t_io16")
            nc.scalar.copy(s1_dr[:], s1_ps[:])
            nc.sync.dma_start(cc_in[1][:], s1_dr[:])
            nc.gpsimd.collective_compute(
                "AllReduce", OP.add, ins=[cc_in[1][:]],
                outs=[cc_out[1][:]], replica_groups=[core_ids])
            s1_all = tail.tile([B, DN], f16, tag="t_io16")
            nc.sync.dma_start(s1_all[:], cc_out[1][:])
            squash_to_outrep(s1_all, orep[1], 1.0)

            # =========== iter 2 (2-stage emission skew) ===========
            s2_ps = psacc.tile([B, DN], f32, tag="sacc")
            it2_tiles = {}
            it2_done = set()
            for step in range(NP + 2):
                pa = step
                if pa < NP:
                    for wpos in range(pa, min(pa + 3, NP)):
                        w = PROD[wpos]
                        if wpos >= RES_P and w not in it2_done \
                                and w not in it2_tiles:
                            it2_tiles[w] = load_pair(w)
                    qa = PROD[pa]
                    if pa < RES_P:
                        u_pt = pair_tiles[qa]
                    else:
                        u_pt = it2_tiles.pop(qa)
                        it2_done.add(qa)
                    it_stage_a(qa, u_pt, orep[1])
                pb = step - 1
                if 0 <= pb < NP:
                    it_stage_b(PROD[pb], first_iter=False)
                pc = step - 2
                if 0 <= pc < NP:
                    emit_fold(it_stage_c(PROD[pc], s2_ps, first_mm=(pc == 0),
                                         last_mm=(pc == NP - 1)))

            s2_dr = tail.tile([B, DN], f32, tag="t_io")
            nc.scalar.copy(s2_dr[:], s2_ps[:])
            nc.sync.dma_start(s2_part[:], s2_dr[:])

    nc.compile()
    return nc


_NC_CACHE = {}


def _get_nc():
    if "nc" not in _NC_CACHE:
        _NC_CACHE["nc"] = _build()
    return _NC_CACHE["nc"]


def _prep_core(x_c, w_c):
    """x_c [B, IL, K] f32, w_c [N, IL, D, K] f32 -> in_map dict."""
    wt = np.ascontiguousarray(w_c.transpose(1, 3, 2, 0))  # [IL, K, D, N]
    wt2 = wt.reshape(NP, 8, K, DN).reshape(NP, 128, DN).astype(np.float16)
    wt4 = np.ascontiguousarray(
        wt2.reshape(NP // 4, 4, 128, DN).transpose(0, 2, 1, 3)
    ).reshape(NP // 4, 128, 4 * DN)
    xt = x_c.transpose(1, 2, 0)  # [IL, K, B]
    x_bd = np.zeros((128, NP, 128), np.float16)
    for g in range(G):
        q, s = g // 2, g % 2
        for j in range(4):
            i = 4 * g + j
            x_bd[s * 64 + j * 16:s * 64 + j * 16 + K, q,
                 j * 32:j * 32 + 32] = xt[i].astype(np.float16)
    # x8[(i8,k), gp, b] = x[b, i, k] for i = gp*8 + i8
    x8 = np.ascontiguousarray(
        xt.reshape(NP, 8 * K, B).transpose(1, 0, 2)).astype(np.float16)
    return {"w_t4": wt4, "x_bd": x_bd, "x8": x8}


def _squash_np(v):
    sn = np.sum(v * v, axis=-1, keepdims=True)
    return np.sqrt(sn) / (1.0 + sn) * v


def _run(inputs, W, trace=False):
    _install_ntff_hook()
    nc = _get_nc()
    x = np.asarray(inputs, np.float32)
    Wf = np.asarray(W, np.float32)
    in_maps = []
    for c in range(NCORES):
        sl = slice(c * IL, (c + 1) * IL)
        in_maps.append(_prep_core(x[:, sl, :], Wf[:, sl, :, :]))
    res = run_bass_kernel_spmd(nc, in_maps, list(range(NCORES)), trace=trace)
    s2 = np.zeros((B, DN), np.float64)
    for c in range(NCORES):
        s2 += res.results[c]["s2_part"].astype(np.float64)
    s2 = s2.reshape(B, D, N).transpose(0, 2, 1).astype(np.float32)
    out = _squash_np(s2).astype(np.float32)
    return out, res


def kernel(inputs, W):
    out, _ = _run(inputs, W, trace=False)
    return out


# revision 38
# speedup vs baseline: 1.0924x; 1.0924x over previous
"""Capsule-routing kernel for 8 Trainium2 NeuronCores — v2.

Problem: u_hat = einsum('nidk,bik->bnid', W, x); 3 rounds of dynamic
routing (softmax over n, weighted sum over i, squash, agreement update).

Sharding: input-capsule axis i (2048) split 8 ways -> 256 i per core.

v2 design (vs baseline 818us):
  - s0 (iteration-0 weighted sum, uniform c) computed DIRECTLY from x,W
    on PE: s0[b,(d,n)] = sum_{i,k} x8[(i8,k),b] * W[(i8,k),(d,n)],
    accumulated over 32 group-pairs in one psum tile. This makes out0
    available ~50us in, long before u_hat materialization completes.
  - W is streamed twice (pass 1 for s0, pass 2 for u_hat); u_hat is kept
    fp16 SBUF-resident for the first RES_P pairs, spilled to DRAM for
    the rest (store once, reload per routing sweep).
  - Sweep-1 u_hat production is software-pipelined WITH routing iter 1
    in a single merged emission loop (per-engine program order matters);
    iter-1 consumes pairs in production order.
  - psum drains rotate over ACT/GPSIMD/DVE; the d-reduction tree is
    split GPS(d32->4)/DVE(d4->1); Z comes free from exp via ACT
    accum_out; e is broadcast (stride-0 middle axis) into the sm-mul so
    no crep materialization.
  - Layouts: u16 partition p = 32*j + b (j = i mod 4 in group), free
    (d,n) d-major. Fold over the 4 j-slots + 1/Z via cz4 ones-matmul
    accumulated across all 64 groups in one psum tile (baseline trick).
"""
import sys
import types
from collections import defaultdict

sys.path.insert(0, "/opt/trn_rl_repo")

import numpy as np

from concourse import bacc, tile, mybir
from concourse.bass_utils import run_bass_kernel_spmd

f32 = mybir.dt.float32
f16 = mybir.dt.float16
AX = mybir.AxisListType
OP = mybir.AluOpType
AF = mybir.ActivationFunctionType

B, N, I, D, K = 32, 64, 2048, 32, 16
NCORES = 8
IL = I // NCORES          # 256 local input capsules
G = IL // 4               # 64 groups of 4 i
NP = G // 2               # 32 group-pairs
DN = D * N                # 2048 free elements per group, d-major
INV_LOG2 = float(1.0 / np.log(2.0))

RES_P = 2                # pairs kept SBUF-resident; rest spilled to DRAM
LAG = 6                   # mega-loop emission lag: iter1 pair q at step q+LAG


def _install_ntff_hook():
    if "antenv.axon_hooks" in sys.modules:
        return
    try:
        mod = types.ModuleType("antenv.axon_hooks")
        state = {"hook": None}
        mod.set_axon_ntff_profile_hook = lambda h: state.__setitem__("hook", h)
        mod.get_axon_ntff_profile_hook = lambda: state["hook"]
        sys.modules["antenv.axon_hooks"] = mod
        import antenv
        antenv.axon_hooks = mod
        from trn_agent_boot.trn_boot import _ntff_profile_via_ctypes
        mod.set_axon_ntff_profile_hook(
            _ntff_profile_via_ctypes("/opt/axon/libaxon_pjrt.so"))
    except Exception:
        pass


def _build():
    nc = bacc.Bacc("TRN2", target_bir_lowering=False, debug=False,
                   num_devices=NCORES)

    w_t2 = nc.dram_tensor("w_t2", [NP, 128, DN], f16, kind="ExternalInput")
    x_bd = nc.dram_tensor("x_bd", [128, NP, 128], f16, kind="ExternalInput")
    x8 = nc.dram_tensor("x8", [128, NP, B], f16, kind="ExternalInput")
    s2_part = nc.dram_tensor("s2_part", [B, DN], f32, kind="ExternalOutput")

    u_spill = nc.dram_tensor("u_spill", [G, 128, DN], f16)
    cc_in = [nc.dram_tensor(f"cc_in{r}", [B, DN], f16) for r in range(2)]
    cc_out = [nc.dram_tensor(f"cc_out{r}", [B, DN], f16, addr_space="Shared")
              for r in range(2)]

    ones4_np = np.zeros((128, 32), np.float16)
    for p in range(128):
        ones4_np[p, p % 32] = 1.0
    ones4 = nc.inline_tensor(ones4_np, name="ones4")

    core_ids = list(range(NCORES))

    with tile.TileContext(nc) as tc:
        with tc.tile_pool(name="const", bufs=1) as constp, \
             tc.tile_pool(name="tail", bufs=1) as tail, \
             tc.tile_pool(name="xw", bufs=1) as xw, \
             tc.tile_pool(name="wp", bufs=6) as wp, \
             tc.tile_pool(name="ures", bufs=1) as ures, \
             tc.tile_pool(name="spo", bufs=2) as spo, \
             tc.tile_pool(name="win", bufs=3) as win, \
             tc.tile_pool(name="tmpp", bufs=2) as tmpp, \
             tc.tile_pool(name="smp", bufs=2) as smp, \
             tc.tile_pool(name="tr1p", bufs=2) as tr1p, \
             tc.tile_pool(name="tr2p", bufs=2) as tr2p, \
             tc.tile_pool(name="tr3p", bufs=2) as tr3p, \
             tc.tile_pool(name="t4p", bufs=2) as t4p, \
             tc.tile_pool(name="bstate", bufs=1) as bstate, \
             tc.tile_pool(name="small", bufs=4) as small, \
             tc.tile_pool(name="psu", bufs=2, space="PSUM") as psu, \
             tc.tile_pool(name="psacc", bufs=1, space="PSUM") as psacc:

            ones_sb = constp.tile([128, 32], f16)
            nc.sync.dma_start(ones_sb[:], ones4[:])
            xbd_sb = xw.tile([128, NP, 128], f16)
            nc.sync.dma_start(xbd_sb[:], x_bd[:])
            x8_sb = xw.tile([128, NP, B], f16)
            nc.scalar.dma_start(x8_sb[:], x8[:])
            orep = [constp.tile([128, DN], f16, tag="orep", name=f"orep{r}")
                    for r in range(2)]

            def squash_to_outrep(s_sb, orep_t, pre_scale):
                """orep [128,(d,n)] f16 <- x4-replicated squash(s_sb*pre_scale)."""
                ps2 = float(pre_scale * pre_scale)
                sq = tail.tile([32, D, N], f32, tag="t_sq")
                nc.scalar.square(sq[:],
                                 s_sb[:].rearrange("p (d n) -> p d n", n=N))
                cur, d = sq, D
                while d > 2:
                    nxt = tail.tile([32, d // 2, N], f32, tag=f"t_tr{d}")
                    nc.vector.tensor_add(nxt[:], cur[:, 0:d // 2, :],
                                         cur[:, d // 2:d, :])
                    cur, d = nxt, d // 2
                sn = tail.tile([32, 1, N], f32, tag="t_sn")
                nc.vector.tensor_add(sn[:], cur[:, 0:1, :], cur[:, 1:2, :])
                r_ = tail.tile([32, N], f32, tag="t_r")
                nc.scalar.activation(r_[:], sn[:, 0, :], AF.Sqrt,
                                     bias=0.0, scale=ps2)
                den = tail.tile([32, N], f32, tag="t_den")
                nc.vector.tensor_scalar(den[:], sn[:, 0, :], ps2, 1.0,
                                        OP.mult, OP.add)
                rd = tail.tile([32, N], f32, tag="t_rd")
                nc.vector.reciprocal(rd[:], den[:])
                fac = tail.tile([32, N], f32, tag="t_fac")
                nc.vector.scalar_tensor_tensor(fac[:], r_[:],
                                               float(pre_scale * INV_LOG2),
                                               rd[:],
                                               op0=OP.mult, op1=OP.mult)
                frep = tail.tile([32, D, N], f16, tag="t_frep")
                nc.scalar.copy(frep[:, 0:1, :], fac[:].unsqueeze(1))
                d = 1
                while d < D:
                    nc.scalar.copy(frep[:, d:2 * d, :], frep[:, 0:d, :])
                    d *= 2
                o16 = tail.tile([32, D, N], f16, tag="t_o16")
                nc.vector.tensor_mul(
                    o16[:], s_sb[:].rearrange("p (d n) -> p d n", n=N), frep[:])
                for j in range(4):
                    nc.sync.dma_start(
                        orep_t[32 * j:32 * j + 32, :],
                        o16[:].rearrange("p d n -> p (d n)"))

            def dma4(dst, src, n=4):
                # split a large DMA along the partition axis so it spreads
                # over n queues instead of serializing on one (~22GB/s each)
                p = dst.shape[0]
                c = p // n
                for a in range(n):
                    nc.sync.dma_start(dst[c * a:c * (a + 1)],
                                      src[c * a:c * (a + 1)])

            # =========== pass 1: s0 directly from x,W ===========
            s0_ps = psacc.tile([B, DN], f32, tag="sacc")
            for gp in range(NP):
                wt = wp.tile([128, DN], f16, tag="wt")
                nc.sync.dma_start(wt[:], w_t2[gp])
                for ch in range(4):
                    nc.tensor.matmul(
                        s0_ps[:, 512 * ch:512 * (ch + 1)],
                        lhsT=x8_sb[:, gp, :],
                        rhs=wt[:, 512 * ch:512 * (ch + 1)],
                        start=(gp == 0), stop=(gp == NP - 1),
                        skip_group_check=True)
            s0_dr = tail.tile([B, DN], f16, tag="t_io16")
            nc.scalar.copy(s0_dr[:], s0_ps[:])
            nc.sync.dma_start(cc_in[0][:], s0_dr[:])
            nc.gpsimd.collective_compute(
                "AllReduce", OP.add, ins=[cc_in[0][:]],
                outs=[cc_out[0][:]], replica_groups=[core_ids])
            s0_all = tail.tile([B, DN], f16, tag="t_io16")
            nc.sync.dma_start(s0_all[:], cc_out[0][:])
            squash_to_outrep(s0_all, orep[0], 1.0 / 64.0)

            # =========== merged: u_hat production (pass 2) + iter 1 ===========
            PROD = list(range(NP))
            RES_SET = set(PROD[:RES_P])
            pair_tiles = {}    # pair id -> resident sbuf tile [128, 2, DN]
            bs_tiles = {}

            def drain(dst, src):
                nc.scalar.copy(dst, src)

            def emit_produce(gp):
                """Pass-2 W load + u_hat matmuls + drains for pair gp."""
                wt = wp.tile([128, DN], f16, tag="wt")
                nc.sync.dma_start(wt[:], w_t2[gp])
                if gp in RES_SET:
                    pt = ures.tile([128, 2, DN], f16, tag=f"ur{gp}",
                                   name=f"ur{gp}")
                else:
                    pt = spo.tile([128, 2, DN], f16, tag="spo")
                pair_tiles[gp] = pt
                for gs in range(2):
                    for h in range(2):
                        pu = psu.tile([128, DN // 2], f32)
                        for ch in range(2):
                            nc.tensor.matmul(
                                pu[:, 512 * ch:512 * (ch + 1)],
                                lhsT=xbd_sb[64 * gs:64 * (gs + 1), gp, :],
                                rhs=wt[64 * gs:64 * (gs + 1),
                                       1024 * h + 512 * ch:
                                       1024 * h + 512 * (ch + 1)],
                                start=True, stop=True)
                        drain(pt[:, gs, 1024 * h:1024 * (h + 1)], pu[:])
                if gp not in RES_SET:
                    nc.sync.dma_start(
                        u_spill[2 * gp:2 * gp + 2].transpose([1, 0, 2]), pt[:])

            def load_pair(q):
                """DRAM -> win tile for spilled pair q."""
                pt = win.tile([128, 2, DN], f16, tag="win")
                nc.sync.dma_start(
                    pt[:], u_spill[2 * q:2 * q + 2].transpose([1, 0, 2]))
                return pt

            # per-pair iter state carried between pipeline stages
            st = {}

            def it_stage_a(q, u_pt, orep_t):
                """tmp mul + DVE tree stages (d 32 -> 4). Plain tensor_tensor
                runs at the 2x DVE rate for packed fp16 — measured faster
                than both GPSIMD (fixed ~3us/op) and STT (~1.3x)."""
                u3 = u_pt[:]                                  # [128, 2, 2048]
                orep3 = orep_t[:].unsqueeze(1).broadcast_to([128, 2, DN])
                tmp = tmpp.tile([128, 2, DN], f16, tag="tmp")
                nc.vector.tensor_mul(tmp[:], u3, orep3)
                t1 = tr1p.tile([128, 2, 16 * N], f16, tag="t1")
                nc.vector.tensor_add(t1[:], tmp[:, :, 0:16 * N],
                                     tmp[:, :, 16 * N:32 * N])
                t2 = tr2p.tile([128, 2, 8 * N], f16, tag="t2")
                nc.vector.tensor_add(t2[:], t1[:, :, 0:8 * N],
                                     t1[:, :, 8 * N:16 * N])
                t3 = tr3p.tile([128, 2, 4 * N], f16, tag="t3")
                nc.vector.tensor_add(t3[:], t2[:, :, 0:4 * N],
                                     t2[:, :, 4 * N:8 * N])
                st[q] = {"u3": u3, "t3": t3}

            def it_stage_b(q, first_iter):
                """DVE tree tail + bs update + max; GPS nm2; ACT exp+Z."""
                t3 = st[q]["t3"]
                t4 = t4p.tile([128, 2, 2 * N], f16, tag="t4")
                nc.vector.tensor_add(t4[:], t3[:, :, 0:2 * N],
                                     t3[:, :, 2 * N:4 * N])
                if first_iter:
                    bs = bstate.tile([128, 2, N], f32, tag=f"bs{q}",
                                     name=f"bs{q}")
                    bs_tiles[q] = bs
                    nc.vector.tensor_add(bs[:], t4[:, :, 0:N], t4[:, :, N:2 * N])
                else:
                    bs = bs_tiles[q]
                    a2 = small.tile([128, 2, N], f32, tag="a2")
                    nc.vector.tensor_add(a2[:], t4[:, :, 0:N], t4[:, :, N:2 * N])
                    nc.vector.tensor_add(bs[:], bs[:], a2[:])
                nm2 = small.tile([128, 2, 1], f32, tag="nm2")
                nc.vector.tensor_reduce(out=nm2[:], in_=bs[:], axis=AX.X,
                                        op=OP.max, negate=True)
                e2 = small.tile([128, 2, N], f16, tag="e2")
                for gs in range(2):
                    nc.scalar.activation(e2[:, gs, :], bs[:, gs, :], AF.Exp,
                                         bias=nm2[:, gs, :], scale=1.0)
                st[q].update(e2=e2)

            def it_stage_c(q, s_ps, first_mm, last_mm):
                """DVE rz + sm mul; ACT cz4; PE folds."""
                d = st.pop(q)
                z2 = small.tile([128, 2, 1], f32, tag="z2")
                nc.vector.tensor_reduce(out=z2[:], in_=d["e2"][:], axis=AX.X,
                                        op=OP.add)
                rz = small.tile([128, 2, 1], f32, tag="rz")
                nc.vector.reciprocal(rz[:], z2[:])
                cz4 = small.tile([128, 2, 32], f16, tag="cz4")
                for gs in range(2):
                    nc.scalar.activation(cz4[:, gs, :], ones_sb[:], AF.Copy,
                                         bias=0.0, scale=rz[:, gs, :])
                sm = smp.tile([128, 2, DN], f16, tag="sm")
                u4 = d["u3"].rearrange("p a (d n) -> p a d n", n=N)
                e4 = d["e2"][:].unsqueeze(2).broadcast_to([128, 2, D, N])
                nc.vector.tensor_mul(
                    sm[:].rearrange("p a (d n) -> p a d n", n=N), u4, e4)
                smf = sm[:]
                return (cz4, smf, first_mm, last_mm, s_ps)

            def emit_fold(fold_args):
                cz4, smf, first_mm, last_mm, s_ps = fold_args
                for gs in range(2):
                    for ch in range(4):
                        nc.tensor.matmul(
                            s_ps[:, 512 * ch:512 * (ch + 1)],
                            lhsT=cz4[:, gs, :],
                            rhs=smf[:, gs, 512 * ch:512 * (ch + 1)],
                            start=(first_mm and gs == 0),
                            stop=(last_mm and gs == 1),
                            skip_group_check=True)

            # fold emission slots: keep PE stream aligned with real time
            fold_slot = {}
            for q in range(NP):
                fold_slot[q] = max(q + LAG + 2, 9 + (3 * q) // 2)
            slot_to_q = defaultdict(list)
            for q, s in fold_slot.items():
                slot_to_q[s].append(q)

            s1_ps = psacc.tile([B, DN], f32, tag="sacc")
            pending_folds = {}
            it1_tiles = {}
            it1_done = set()
            n_steps = max(NP, max(fold_slot.values()) + 1)
            for step in range(n_steps):
                pa = step - LAG
                if 0 <= pa < NP:
                    for wpos in range(pa, min(pa + 3, NP)):
                        w = PROD[wpos]
                        if wpos >= RES_P and w not in it1_done \
                                and w not in it1_tiles:
                            it1_tiles[w] = load_pair(w)
                    qa = PROD[pa]
                    if pa < RES_P:
                        u_pt = pair_tiles[qa]
                    else:
                        u_pt = it1_tiles.pop(qa)
                        it1_done.add(qa)
                    it_stage_a(qa, u_pt, orep[0])
                pb = step - LAG - 1
                if 0 <= pb < NP:
                    it_stage_b(PROD[pb], first_iter=True)
                pc = step - LAG - 2
                if 0 <= pc < NP:
                    pending_folds[pc] = it_stage_c(
                        PROD[pc], s1_ps, first_mm=(pc == 0),
                        last_mm=(pc == NP - 1))
                if step < NP:
                    emit_produce(PROD[step])
                for p_ in slot_to_q.get(step, ()):
                    emit_fold(pending_folds.pop(p_))

            # iter-1 tail: AllReduce + squash -> orep1
            s1_dr = tail.tile([B, DN], f16, tag="t_io16")
            nc.scalar.copy(s1_dr[:], s1_ps[:])
            nc.sync.dma_start(cc_in[1][:], s1_dr[:])
            nc.gpsimd.collective_compute(
                "AllReduce", OP.add, ins=[cc_in[1][:]],
                outs=[cc_out[1][:]], replica_groups=[core_ids])
            s1_all = tail.tile([B, DN], f16, tag="t_io16")
            nc.sync.dma_start(s1_all[:], cc_out[1][:])
            squash_to_outrep(s1_all, orep[1], 1.0)

            # =========== iter 2 (2-stage emission skew) ===========
            s2_ps = psacc.tile([B, DN], f32, tag="sacc")
            it2_tiles = {}
            it2_done = set()
            for step in range(NP + 2):
                pa = step
                if pa < NP:
                    for wpos in range(pa, min(pa + 3, NP)):
                        w = PROD[wpos]
                        if wpos >= RES_P and w not in it2_done \
                                and w not in it2_tiles:
                            it2_tiles[w] = load_pair(w)
                    qa = PROD[pa]
                    if pa < RES_P:
                        u_pt = pair_tiles[qa]
                    else:
                        u_pt = it2_tiles.pop(qa)
                        it2_done.add(qa)
                    it_stage_a(qa, u_pt, orep[1])
                pb = step - 1
                if 0 <= pb < NP:
                    it_stage_b(PROD[pb], first_iter=False)
                pc = step - 2
                if 0 <= pc < NP:
                    emit_fold(it_stage_c(PROD[pc], s2_ps, first_mm=(pc == 0),
                                         last_mm=(pc == NP - 1)))

            s2_dr = tail.tile([B, DN], f32, tag="t_io")
            nc.scalar.copy(s2_dr[:], s2_ps[:])
            nc.sync.dma_start(s2_part[:], s2_dr[:])

    nc.compile()
    return nc


_NC_CACHE = {}


def _get_nc():
    if "nc" not in _NC_CACHE:
        _NC_CACHE["nc"] = _build()
    return _NC_CACHE["nc"]


def _prep_core(x_c, w_c):
    """x_c [B, IL, K] f32, w_c [N, IL, D, K] f32 -> in_map dict."""
    wt = np.ascontiguousarray(w_c.transpose(1, 3, 2, 0))  # [IL, K, D, N]
    wt2 = wt.reshape(NP, 8, K, DN).reshape(NP, 128, DN).astype(np.float16)
    xt = x_c.transpose(1, 2, 0)  # [IL, K, B]
    x_bd = np.zeros((128, NP, 128), np.float16)
    for g in range(G):
        q, s = g // 2, g % 2
        for j in range(4):
            i = 4 * g + j
            x_bd[s * 64 + j * 16:s * 64 + j * 16 + K, q,
                 j * 32:j * 32 + 32] = xt[i].astype(np.float16)
    # x8[(i8,k), gp, b] = x[b, i, k] for i = gp*8 + i8
    x8 = np.ascontiguousarray(
        xt.reshape(NP, 8 * K, B).transpose(1, 0, 2)).astype(np.float16)
    return {"w_t2": wt2, "x_bd": x_bd, "x8": x8}


def _squash_np(v):
    sn = np.sum(v * v, axis=-1, keepdims=True)
    return np.sqrt(sn) / (1.0 + sn) * v


def _run(inputs, W, trace=False):
    _install_ntff_hook()
    nc = _get_nc()
    x = np.asarray(inputs, np.float32)
    Wf = np.asarray(W, np.float32)
    in_maps = []
    for c in range(NCORES):
        sl = slice(c * IL, (c + 1) * IL)
        in_maps.append(_prep_core(x[:, sl, :], Wf[:, sl, :, :]))
    res = run_bass_kernel_spmd(nc, in_maps, list(range(NCORES)), trace=trace)
    s2 = np.zeros((B, DN), np.float64)
    for c in range(NCORES):
        s2 += res.results[c]["s2_part"].astype(np.float64)
    s2 = s2.reshape(B, D, N).transpose(0, 2, 1).astype(np.float32)
    out = _squash_np(s2).astype(np.float32)
    return out, res


def kernel(inputs, W):
    out, _ = _run(inputs, W, trace=False)
    return out
